# revision 23
# baseline (speedup 1.0000x reference)
"""Causal GQA self-attention block (B=4, S=2048, D=2048, 16 q-heads / 4 kv-heads)
on 8 Trainium2 NeuronCores.

Sharding: TP2 x DP4. Core c handles batch b = c//2 and head-half h = c%2
(q-heads 8h..8h+7, kv-heads 2h..2h+1). Each core computes a [2048, 2048]
partial of the output projection (transposed, [out_dim, seq]); the host sums
the two TP partials per batch and transposes back.

Per-core pipeline (all matmuls bf16 inputs / fp32 PSUM accumulation):
  1. QKV projection: lhsT = xT tiles (stationary), rhs = fused W^T (moving).
  2. Fused RMS-norm + RoPE + (gain/sqrt(hd)) scaling in natural layout, then
     PE-transpose Q,K head tiles to [hd, seq]; V kept natural [seq, hd].
  3. Attention per head in transposed layout: S^T[k,q] blocks = K_blk^T Q^T,
     additive causal mask on diagonal-band blocks, plain exp (no max
     subtraction: rms-normed q,k bound |score| <= sqrt(hd)), softmax
     denominator via DVE accumulate + ones-matmul partition reduction,
     unnormalized y^T accumulated in PSUM, normalized at eviction with a
     PE-broadcast reciprocal row.
  4. Output projection: out^T[o, s] accumulated over the core's 8 heads.
"""
import sys

if "/opt/trn_rl_repo" not in sys.path:
    sys.path.insert(0, "/opt/trn_rl_repo")

import numpy as np
import ml_dtypes

import concourse.bass as bass
import concourse.mybir as mybir
from concourse import bacc
from concourse.tile import TileContext
from concourse.bass_utils import run_bass_kernel_spmd
from concourse.masks import make_identity

BF16 = mybir.dt.bfloat16
F32 = mybir.dt.float32
F32R = mybir.dt.float32r
AF = mybir.ActivationFunctionType
OP = mybir.AluOpType

DIM = 2048
SEQ = 2048
BATCH = 4
HD = 128
NH_L = 8            # q heads per core
NKV_L = 2           # kv heads per core
QKV = (NH_L + 2 * NKV_L) * HD   # 1536
N_ST = SEQ // 128   # 16 seq tiles
N_DC = DIM // 128   # 16 contraction chunks
N_QT = SEQ // 512   # 4 query tiles of 512
EPS = 1.1920928955078125e-07
NEG = -1.0e30
N_CORES = 8

_CACHED_NC = None


def _build_nc():
    nc = bacc.Bacc(
        "TRN2",
        target_bir_lowering=False,
        debug=False,
        num_devices=N_CORES,
    )
    xT = nc.dram_tensor("xT", [DIM, SEQ], BF16, kind="ExternalInput")
    wt = nc.dram_tensor("wt", [DIM, QKV], BF16, kind="ExternalInput")
    # wp2[p, ot*NH_L*128 + ic*128 + c] = Wproj[ot*128+c, h*1024 + ic*128 + p]
    wp2 = nc.dram_tensor("wp2", [128, (DIM // 128) * NH_L * 128], BF16,
                         kind="ExternalInput")
    cs = nc.dram_tensor("cs", [SEQ, HD], F32, kind="ExternalInput")
    gains = nc.dram_tensor("gains", [128, NH_L + NKV_L], F32, kind="ExternalInput")
    mtri = nc.dram_tensor("mtri", [128, 128], BF16, kind="ExternalInput")
    out = nc.dram_tensor("out", [DIM, SEQ], F32, kind="ExternalOutput")

    with TileContext(nc) as tc, \
         nc.allow_low_precision(reason="f32r softmax denominators"):
        with tc.tile_pool(name="const", bufs=1) as const, \
             tc.tile_pool(name="persist", bufs=1) as persist:
            ident = const.tile([128, 128], BF16)
            make_identity(nc, ident[:])
            ones_col = const.tile([128, 1], BF16)
            nc.gpsimd.memset(ones_col[:], 1.0)
            eps_sb = const.tile([128, 1], F32)
            nc.gpsimd.memset(eps_sb[:], EPS)
            gains_sb = const.tile([128, NH_L + NKV_L], F32)
            nc.sync.dma_start(gains_sb[:], gains[:])
            mtri_sb = const.tile([128, 128], BF16)
            nc.sync.dma_start(mtri_sb[:], mtri[:])
            wt_sb = const.tile([128, N_DC * QKV], BF16)

            # persistent activations
            qt_sb = persist.tile([128, NH_L * SEQ], BF16)   # Q^T per head
            kt_sb = persist.tile([128, NKV_L * SEQ], BF16)  # K^T per kv head
            v_sb = persist.tile([128, N_ST * NKV_L * HD], BF16)  # V natural
            yt_sb = persist.tile([128, NH_L * SEQ], BF16)   # y^T per head

            # ---------------- stage A: QKV projection + norm/rope/transpose
            with tc.tile_pool(name="a_sbuf", bufs=2) as a_sbuf, \
                 tc.tile_pool(name="a_stat", bufs=2) as a_stat, \
                 tc.tile_pool(name="a_psum", bufs=2, space="PSUM") as a_psum, \
                 tc.tile_pool(name="t_psum", bufs=2, space="PSUM") as t_psum:

                def emit_transposes(st, natq):
                    for j in range(NH_L + NKV_L):
                        tp = t_psum.tile([128, 128], BF16, tag="tp")
                        nc.tensor.transpose(
                            tp[:], natq[:, j * 128:(j + 1) * 128], ident[:])
                        if j < NH_L:
                            dst = qt_sb[:, j * SEQ + st * 128: j * SEQ + (st + 1) * 128]
                        else:
                            jj = j - NH_L
                            dst = kt_sb[:, jj * SEQ + st * 128: jj * SEQ + (st + 1) * 128]
                        nc.scalar.copy(dst, tp[:])

                # xT arrives in 4-seq-tile quads (1KB contiguous runs per
                # partition); quad q+1 is prefetched while quad q computes.
                # The first quad + rope tables are issued BEFORE the big
                # weight DMA so the PE can start almost immediately.
                def load_quad(q):
                    t = a_sbuf.tile([128, N_DC * 512], BF16, tag="xt")
                    for c in range(N_DC):
                        nc.sync.dma_start(
                            t[:, c * 512:(c + 1) * 512],
                            xT[c * 128:(c + 1) * 128, q * 512:(q + 1) * 512])
                    ct = a_sbuf.tile([128, 4 * HD], F32, tag="cs")
                    nc.sync.dma_start(
                        ct[:].rearrange("p (st d) -> p st d", st=4),
                        cs[q * 512:(q + 1) * 512, :].rearrange(
                            "(st p) d -> p st d", p=128))
                    return t, ct

                quads = [None] * 4
                quads[0] = load_quad(0)
                for c in range(N_DC):
                    nc.sync.dma_start(
                        wt_sb[:, c * QKV:(c + 1) * QKV],
                        wt[c * 128:(c + 1) * 128, :])

                prev = None
                for st in range(N_ST):
                    q, off = st // 4, st % 4
                    if off == 0 and q + 1 < 4:
                        quads[q + 1] = load_quad(q + 1)
                    xq, ctq = quads[q]
                    cs_t = ctq[:, off * HD:(off + 1) * HD]

                    qkv_ps = a_psum.tile([128, QKV], F32, tag="qkv")
                    for c in range(N_DC):
                        for n in range(QKV // 512):
                            nc.tensor.matmul(
                                qkv_ps[:, n * 512:(n + 1) * 512],
                                xq[:, c * 512 + off * 128:
                                   c * 512 + off * 128 + 128],
                                wt_sb[:, c * QKV + n * 512: c * QKV + (n + 1) * 512],
                                start=(c == 0), stop=(c == N_DC - 1))

                    if prev is not None:
                        emit_transposes(*prev)

                    # rms statistics for the 10 normed heads
                    sq = a_stat.tile([128, 128], F32, tag="sq")
                    ssq = a_stat.tile([128, NH_L + NKV_L], F32, tag="ssq")
                    for j in range(NH_L + NKV_L):
                        nc.scalar.activation(
                            sq[:], qkv_ps[:, j * 128:(j + 1) * 128],
                            AF.Square, accum_out=ssq[:, j:j + 1])
                    rr = a_stat.tile([128, NH_L + NKV_L], F32, tag="rr")
                    nc.scalar.activation(rr[:], ssq[:], AF.Sqrt,
                                         scale=1.0 / HD, bias=eps_sb[:])
                    ri = a_stat.tile([128, NH_L + NKV_L], F32, tag="ri")
                    nc.vector.reciprocal(ri[:], rr[:])
                    rq = a_stat.tile([128, NH_L + NKV_L], F32, tag="rq")
                    nc.vector.tensor_mul(rq[:], ri[:], gains_sb[:])

                    # batched rope (strided APs over all 10 heads), then
                    # per-head scale by rq in place
                    natq = a_sbuf.tile([128, (NH_L + NKV_L) * 128], BF16, tag="natq")
                    nh = NH_L + NKV_L
                    qv = qkv_ps[:, :nh * 128].rearrange(
                        "p (h two s) -> p h two s", h=nh, two=2)
                    nv = natq[:].rearrange(
                        "p (h two s) -> p h two s", h=nh, two=2)
                    u1 = qv[:, :, 0, :]
                    u2 = qv[:, :, 1, :]
                    o1 = nv[:, :, 0, :]
                    o2 = nv[:, :, 1, :]
                    cob = cs_t[:, 0:64].unsqueeze(1).broadcast_to((128, nh, 64))
                    sib = cs_t[:, 64:128].unsqueeze(1).broadcast_to((128, nh, 64))
                    ta = a_stat.tile([128, nh * 64], F32, tag="ta")
                    tb = a_stat.tile([128, nh * 64], F32, tag="tb")
                    tav = ta[:].rearrange("p (h s) -> p h s", h=nh)
                    tbv = tb[:].rearrange("p (h s) -> p h s", h=nh)
                    nc.vector.tensor_mul(tav, u1, cob)
                    nc.vector.tensor_mul(tbv, u2, sib)
                    nc.vector.tensor_add(o1, tav, tbv)
                    nc.vector.tensor_mul(tav, u2, cob)
                    nc.vector.tensor_mul(tbv, u1, sib)
                    nc.vector.tensor_sub(o2, tav, tbv)
                    for j in range(nh):
                        nc.vector.tensor_scalar_mul(
                            natq[:, j * 128:(j + 1) * 128],
                            natq[:, j * 128:(j + 1) * 128], rq[:, j:j + 1])

                    # V eviction (natural layout, bf16)
                    nc.vector.tensor_copy(
                        v_sb[:, st * NKV_L * HD:(st + 1) * NKV_L * HD],
                        qkv_ps[:, NH_L * 128 + NKV_L * 128:])

                    prev = (st, natq)
                emit_transposes(*prev)

            # ---------------- stage B: attention per head
            # S^T blocks with causal column restriction (no additive mask);
            # diagonal-pair P tiles keep permanently-zeroed invalid regions,
            # a bf16 triangle mask handles the exact-diagonal 128x128
            # sub-blocks. Softmax denominators accumulate in PSUM via one
            # ones-column matmul per pair; 1/den rows are broadcast across
            # partitions on the (otherwise idle) GpSimd engine.
            # The stage-C weight pool opens here so the first projection
            # weight tiles stream in during attention.
            c_sbuf_cm = tc.tile_pool(name="c_sbuf", bufs=3)
            c_sbuf = c_sbuf_cm.__enter__()
            wp_pre = []
            for ot in range(3):
                wp_t = c_sbuf.tile([128, NH_L * 128], BF16, tag="wp")
                nc.sync.dma_start(
                    wp_t[:], wp2[:, ot * NH_L * 128:(ot + 1) * NH_L * 128])
                wp_pre.append(wp_t)
            with tc.tile_pool(name="b_sbuf", bufs=3) as b_sbuf, \
                 tc.tile_pool(name="b_diag", bufs=3) as b_diag, \
                 tc.tile_pool(name="b_row", bufs=3) as b_row, \
                 tc.tile_pool(name="s_psum", bufs=2, space="PSUM") as s_psum, \
                 tc.tile_pool(name="y_psum", bufs=3, space="PSUM") as y_psum, \
                 tc.tile_pool(name="d_psum", bufs=1, space="PSUM") as d_psum:
                # pre-zero the never-written regions of the diagonal-pair
                # tiles (ring of 3 per tag); exp only writes the causally
                # valid columns afterwards, so the zeros persist.
                for _ in range(3):
                    t0 = b_diag.tile([128, 1024], BF16, tag="pd0")
                    nc.gpsimd.memset(t0[:, 512:640], 0.0)
                    t1 = b_diag.tile([128, 1024], BF16, tag="pd1")
                    nc.gpsimd.memset(t1[:, 0:256], 0.0)
                    nc.gpsimd.memset(t1[:, 512:896], 0.0)

                for j in range(NH_L):
                    jj = j // (NH_L // NKV_L)
                    for qt in range(N_QT):
                        nblk = 4 * qt + 4
                        npair = nblk // 2
                        q0 = j * SEQ + qt * 512
                        q_sl = slice(q0, q0 + 512)
                        y_ps = y_psum.tile([128, 512], F32, tag="y")
                        dn = d_psum.tile([1, 512], F32, tag="dn")

                        def emit_y(pair):
                            g0, pp = pair
                            for half in range(2):
                                kb = 2 * g0 + half
                                nc.tensor.matmul(
                                    y_ps[:],
                                    v_sb[:, kb * NKV_L * HD + jj * HD:
                                         kb * NKV_L * HD + (jj + 1) * HD],
                                    pp[:, half * 512:(half + 1) * 512],
                                    start=(kb == 0), stop=(kb == nblk - 1))

                        pend = None
                        for g in range(npair):
                            diag = g >= 2 * qt
                            if diag:
                                p_bf = b_diag.tile(
                                    [128, 1024], BF16, tag=f"pd{g - 2 * qt}")
                            else:
                                p_bf = b_sbuf.tile([128, 1024], BF16, tag="p")
                            s_ps = s_psum.tile([128, 1024], F32, tag="s")
                            for half in range(2):
                                kb = 2 * g + half
                                d = kb - 4 * qt
                                vo = max(d, 0) * 128
                                nc.tensor.matmul(
                                    s_ps[:, half * 512 + vo:(half + 1) * 512],
                                    kt_sb[:, jj * SEQ + kb * 128:
                                          jj * SEQ + (kb + 1) * 128],
                                    qt_sb[:, q0 + vo:q0 + 512],
                                    start=True, stop=True)
                            if not diag:
                                nc.scalar.activation(p_bf[:], s_ps[:], AF.Exp)
                            else:
                                for half in range(2):
                                    d = 2 * g + half - 4 * qt
                                    vo = d * 128
                                    sl = slice(half * 512 + vo, (half + 1) * 512)
                                    nc.scalar.activation(
                                        p_bf[:, sl], s_ps[:, sl], AF.Exp)
                                for half in range(2):
                                    d = 2 * g + half - 4 * qt
                                    sl = slice(half * 512 + d * 128,
                                               half * 512 + d * 128 + 128)
                                    nc.vector.tensor_mul(
                                        p_bf[:, sl], p_bf[:, sl], mtri_sb[:])
                            tmp = b_sbuf.tile([128, 512], BF16, tag="tmp")
                            nc.vector.tensor_add(
                                tmp[:], p_bf[:, :512], p_bf[:, 512:])
                            nc.tensor.matmul(
                                dn[:], ones_col[:], tmp[:],
                                start=(g == 0), stop=(g == npair - 1))
                            if pend is not None:
                                emit_y(pend)
                            pend = (g, p_bf)
                        emit_y(pend)

                        rinv = b_row.tile([1, 512], F32, tag="rinv")
                        nc.vector.reciprocal_approx_fast(rinv[:], dn[:])
                        rb_sb = b_sbuf.tile([128, 512], F32, tag="rbs")
                        nc.gpsimd.partition_broadcast(rb_sb[:], rinv[:])
                        nc.vector.tensor_mul(
                            yt_sb[:, q_sl], y_ps[:], rb_sb[:])

            # ---------------- stage C: output projection
            with tc.tile_pool(name="c_psum", bufs=2, space="PSUM") as c_psum:
                for ot in range(DIM // 128):
                    if ot < 3:
                        wp_t = wp_pre[ot]
                    else:
                        wp_t = c_sbuf.tile([128, NH_L * 128], BF16, tag="wp")
                        nc.sync.dma_start(
                            wp_t[:],
                            wp2[:, ot * NH_L * 128:(ot + 1) * NH_L * 128])
                    po_ps = c_psum.tile([128, SEQ], F32, tag="po")
                    for ic in range(NH_L):
                        for sc in range(SEQ // 512):
                            nc.tensor.matmul(
                                po_ps[:, sc * 512:(sc + 1) * 512],
                                wp_t[:, ic * 128:(ic + 1) * 128],
                                yt_sb[:, ic * SEQ + sc * 512: ic * SEQ + (sc + 1) * 512],
                                start=(ic == 0), stop=(ic == NH_L - 1))
                    o_sb = c_sbuf.tile([128, SEQ], F32, tag="osb")
                    for sc in range(SEQ // 512):
                        nc.scalar.copy(
                            o_sb[:, sc * 512:(sc + 1) * 512],
                            po_ps[:, sc * 512:(sc + 1) * 512])
                    nc.sync.dma_start(out[ot * 128:(ot + 1) * 128, :], o_sb[:])
            c_sbuf_cm.__exit__(None, None, None)

    nc.compile()
    return nc


def _get_nc():
    global _CACHED_NC
    if _CACHED_NC is None:
        _CACHED_NC = _build_nc()
    return _CACHED_NC


def _make_rope_tables():
    inv_freq = 1.0 / (10000.0 ** (np.arange(0, HD, 2, dtype=np.float32) / HD))
    t = np.arange(SEQ, dtype=np.float32)
    freqs = np.outer(t, inv_freq)
    return np.concatenate(
        [np.cos(freqs), np.sin(freqs)], axis=1).astype(np.float32)


def _make_mtri():
    # lower-triangle-inclusive binary mask for the exact-diagonal block:
    # valid when query column >= key partition
    col = np.arange(128)[None, :]
    row = np.arange(128)[:, None]
    return np.ascontiguousarray(
        (col >= row).astype(ml_dtypes.bfloat16))


def _prep_in_maps(x, Wq, Wk, Wv, Wproj, q_gain):
    x = np.asarray(x, dtype=np.float32)
    Wq = np.asarray(Wq, dtype=np.float32)
    Wk = np.asarray(Wk, dtype=np.float32)
    Wv = np.asarray(Wv, dtype=np.float32)
    Wproj = np.asarray(Wproj, dtype=np.float32)
    q_gain = np.asarray(q_gain, dtype=np.float32)

    bf = ml_dtypes.bfloat16
    cs = _make_rope_tables()
    mtri = _make_mtri()
    xT = [np.ascontiguousarray(x[b].T).astype(bf) for b in range(BATCH)]
    wt_h, wp_h, g_h = [], [], []
    for h in range(2):
        w = np.concatenate([
            Wq[1024 * h:1024 * (h + 1)],
            Wk[256 * h:256 * (h + 1)],
            Wv[256 * h:256 * (h + 1)]], axis=0)
        wt_h.append(np.ascontiguousarray(w.T).astype(bf))
        wpT = Wproj[:, 1024 * h:1024 * (h + 1)].T  # [1024, 2048]
        wp_h.append(np.ascontiguousarray(
            wpT.reshape(NH_L, 128, DIM // 128, 128)
               .transpose(1, 2, 0, 3)
               .reshape(128, (DIM // 128) * NH_L * 128)).astype(bf))
        g = np.concatenate([
            q_gain[8 * h:8 * (h + 1)] / np.sqrt(HD),
            np.ones(NKV_L, np.float32)]).astype(np.float32)
        g_h.append(np.ascontiguousarray(
            np.broadcast_to(g[None, :], (128, NH_L + NKV_L))))

    in_maps = []
    for c in range(N_CORES):
        b, h = c // 2, c % 2
        in_maps.append({
            "xT": xT[b], "wt": wt_h[h], "wp2": wp_h[h], "cs": cs,
            "gains": g_h[h], "mtri": mtri,
        })
    return in_maps


def kernel(x, Wq, Wk, Wv, Wproj, q_gain):
    in_maps = _prep_in_maps(x, Wq, Wk, Wv, Wproj, q_gain)
    nc = _get_nc()
    res = run_bass_kernel_spmd(nc, in_maps, list(range(N_CORES))).results

    out = np.empty((BATCH, SEQ, DIM), dtype=np.float32)
    for b in range(BATCH):
        out[b] = (res[2 * b]["out"] + res[2 * b + 1]["out"]).T
    return out



# revision 27
# speedup vs baseline: 1.0694x; 1.0694x over previous
"""Causal GQA self-attention block (B=4, S=2048, D=2048, 16 q-heads / 4 kv-heads)
on 8 Trainium2 NeuronCores.

Sharding: TP2 x DP4. Core c handles batch b = c//2 and head-half h = c%2
(q-heads 8h..8h+7, kv-heads 2h..2h+1). Each core computes a [2048, 2048]
partial of the output projection (transposed, [out_dim, seq]); the host sums
the two TP partials per batch and transposes back.

Per-core pipeline (all matmuls bf16 inputs / fp32 PSUM accumulation):
  1. QKV projection: lhsT = xT tiles (stationary), rhs = fused W^T (moving).
  2. Fused RMS-norm + RoPE + (gain/sqrt(hd)) scaling in natural layout, then
     PE-transpose Q,K head tiles to [hd, seq]; V kept natural [seq, hd].
  3. Attention per head in transposed layout: S^T[k,q] blocks = K_blk^T Q^T,
     additive causal mask on diagonal-band blocks, plain exp (no max
     subtraction: rms-normed q,k bound |score| <= sqrt(hd)), softmax
     denominator via DVE accumulate + ones-matmul partition reduction,
     unnormalized y^T accumulated in PSUM, normalized at eviction with a
     PE-broadcast reciprocal row.
  4. Output projection: out^T[o, s] accumulated over the core's 8 heads.
"""
import sys

if "/opt/trn_rl_repo" not in sys.path:
    sys.path.insert(0, "/opt/trn_rl_repo")

import numpy as np
import ml_dtypes

import concourse.bass as bass
import concourse.mybir as mybir
from concourse import bacc
from concourse.tile import TileContext
from concourse.bass_utils import run_bass_kernel_spmd
from concourse.masks import make_identity

BF16 = mybir.dt.bfloat16
F32 = mybir.dt.float32
F32R = mybir.dt.float32r
AF = mybir.ActivationFunctionType
OP = mybir.AluOpType

DIM = 2048
SEQ = 2048
BATCH = 4
HD = 128
NH_L = 8            # q heads per core
NKV_L = 2           # kv heads per core
QKV = (NH_L + 2 * NKV_L) * HD   # 1536
N_ST = SEQ // 128   # 16 seq tiles
N_DC = DIM // 128   # 16 contraction chunks
N_QT = SEQ // 512   # 4 query tiles of 512
EPS = 1.1920928955078125e-07
NEG = -1.0e30
N_CORES = 8

_CACHED_NC = None


def _build_nc():
    nc = bacc.Bacc(
        "TRN2",
        target_bir_lowering=False,
        debug=False,
        num_devices=N_CORES,
    )
    xT = nc.dram_tensor("xT", [DIM, SEQ], BF16, kind="ExternalInput")
    wt = nc.dram_tensor("wt", [DIM, QKV], BF16, kind="ExternalInput")
    # wp2[p, ot*NH_L*128 + ic*128 + c] = Wproj[ot*128+c, h*1024 + ic*128 + p]
    wp2 = nc.dram_tensor("wp2", [128, (DIM // 128) * NH_L * 128], BF16,
                         kind="ExternalInput")
    cs = nc.dram_tensor("cs", [SEQ, HD], F32, kind="ExternalInput")
    gains = nc.dram_tensor("gains", [128, NH_L + NKV_L], F32, kind="ExternalInput")
    mtri = nc.dram_tensor("mtri", [128, 128], BF16, kind="ExternalInput")
    out = nc.dram_tensor("out", [DIM, SEQ], F32, kind="ExternalOutput")

    with TileContext(nc) as tc, \
         nc.allow_low_precision(reason="f32r softmax denominators"):
        with tc.tile_pool(name="const", bufs=1) as const, \
             tc.tile_pool(name="persist", bufs=1) as persist:
            ident = const.tile([128, 128], BF16)
            make_identity(nc, ident[:])
            ones_col = const.tile([128, 1], BF16)
            nc.gpsimd.memset(ones_col[:], 1.0)
            eps_sb = const.tile([128, 1], F32)
            nc.gpsimd.memset(eps_sb[:], EPS)
            gains_sb = const.tile([128, NH_L + NKV_L], F32)
            nc.sync.dma_start(gains_sb[:], gains[:])
            mtri_sb = const.tile([128, 128], BF16)
            nc.sync.dma_start(mtri_sb[:], mtri[:])
            # pre-warm the gpsimd partition_broadcast ucode so the first
            # real broadcast in stage B doesn't pay the load stall
            pb_in = const.tile([1, 8], F32)
            nc.gpsimd.memset(pb_in[:], 1.0)
            pb_out = const.tile([128, 8], F32)
            nc.gpsimd.partition_broadcast(pb_out[:], pb_in[:])
            wt_sb = const.tile([128, N_DC * QKV], BF16)

            # persistent activations
            qt_sb = persist.tile([128, NH_L * SEQ], BF16)   # Q^T per head
            kt_sb = persist.tile([128, NKV_L * SEQ], BF16)  # K^T per kv head
            v_sb = persist.tile([128, N_ST * NKV_L * HD], BF16)  # V natural
            yt_sb = persist.tile([128, NH_L * SEQ], BF16)   # y^T per head

            # ---------------- stage A: QKV projection + norm/rope/transpose
            with tc.tile_pool(name="a_sbuf", bufs=2) as a_sbuf, \
                 tc.tile_pool(name="a_stat", bufs=2) as a_stat, \
                 tc.tile_pool(name="a_psum", bufs=2, space="PSUM") as a_psum, \
                 tc.tile_pool(name="t_psum", bufs=2, space="PSUM") as t_psum:

                def emit_transposes(st, natq):
                    for j in range(NH_L + NKV_L):
                        tp = t_psum.tile([128, 128], BF16, tag="tp")
                        nc.tensor.transpose(
                            tp[:], natq[:, j * 128:(j + 1) * 128], ident[:])
                        if j < NH_L:
                            dst = qt_sb[:, j * SEQ + st * 128: j * SEQ + (st + 1) * 128]
                        else:
                            jj = j - NH_L
                            dst = kt_sb[:, jj * SEQ + st * 128: jj * SEQ + (st + 1) * 128]
                        nc.scalar.copy(dst, tp[:])

                # xT arrives in 4-seq-tile quads (1KB contiguous runs per
                # partition); quad q+1 is prefetched while quad q computes.
                # The first quad + rope tables are issued BEFORE the big
                # weight DMA so the PE can start almost immediately.
                def load_quad(q):
                    t = a_sbuf.tile([128, N_DC * 512], BF16, tag="xt")
                    for c in range(N_DC):
                        nc.sync.dma_start(
                            t[:, c * 512:(c + 1) * 512],
                            xT[c * 128:(c + 1) * 128, q * 512:(q + 1) * 512])
                    ct = a_sbuf.tile([128, 4 * HD], F32, tag="cs")
                    nc.sync.dma_start(
                        ct[:].rearrange("p (st d) -> p st d", st=4),
                        cs[q * 512:(q + 1) * 512, :].rearrange(
                            "(st p) d -> p st d", p=128))
                    return t, ct

                quads = [None] * 4
                quads[0] = load_quad(0)
                # weight load rides the Activation HWDGE queue so it streams
                # in parallel with the xT quad on the SP queue
                for c in range(N_DC):
                    nc.scalar.dma_start(
                        wt_sb[:, c * QKV:(c + 1) * QKV],
                        wt[c * 128:(c + 1) * 128, :])

                prev = None
                for st in range(N_ST):
                    q, off = st // 4, st % 4
                    if off == 0 and q + 1 < 4:
                        quads[q + 1] = load_quad(q + 1)
                    xq, ctq = quads[q]
                    cs_t = ctq[:, off * HD:(off + 1) * HD]

                    qkv_ps = a_psum.tile([128, QKV], F32, tag="qkv")
                    for c in range(N_DC):
                        for n in range(QKV // 512):
                            nc.tensor.matmul(
                                qkv_ps[:, n * 512:(n + 1) * 512],
                                xq[:, c * 512 + off * 128:
                                   c * 512 + off * 128 + 128],
                                wt_sb[:, c * QKV + n * 512: c * QKV + (n + 1) * 512],
                                start=(c == 0), stop=(c == N_DC - 1))

                    if prev is not None:
                        emit_transposes(*prev)

                    # rms statistics for the 10 normed heads: one batched
                    # square + one 3D free-axis reduce (vs 10 accum_out
                    # activations, whose accumulator reads serialized ACT)
                    nh = NH_L + NKV_L
                    sq = a_stat.tile([128, nh * 128], BF16, tag="sq")
                    nc.scalar.activation(sq[:], qkv_ps[:, :nh * 128], AF.Square)
                    ssq = a_stat.tile([128, nh], F32, tag="ssq")
                    nc.vector.tensor_reduce(
                        ssq[:], sq[:].rearrange("p (h d) -> p h d", h=nh),
                        mybir.AxisListType.X, OP.add)
                    rr = a_stat.tile([128, NH_L + NKV_L], F32, tag="rr")
                    nc.scalar.activation(rr[:], ssq[:], AF.Sqrt,
                                         scale=1.0 / HD, bias=eps_sb[:])
                    ri = a_stat.tile([128, NH_L + NKV_L], F32, tag="ri")
                    nc.vector.reciprocal(ri[:], rr[:])
                    rq = a_stat.tile([128, NH_L + NKV_L], F32, tag="rq")
                    nc.vector.tensor_mul(rq[:], ri[:], gains_sb[:])

                    # batched rope (strided APs over all 10 heads), then
                    # per-head scale by rq in place
                    natq = a_sbuf.tile([128, (NH_L + NKV_L) * 128], BF16, tag="natq")
                    nh = NH_L + NKV_L
                    qv = qkv_ps[:, :nh * 128].rearrange(
                        "p (h two s) -> p h two s", h=nh, two=2)
                    nv = natq[:].rearrange(
                        "p (h two s) -> p h two s", h=nh, two=2)
                    u1 = qv[:, :, 0, :]
                    u2 = qv[:, :, 1, :]
                    o1 = nv[:, :, 0, :]
                    o2 = nv[:, :, 1, :]
                    cob = cs_t[:, 0:64].unsqueeze(1).broadcast_to((128, nh, 64))
                    sib = cs_t[:, 64:128].unsqueeze(1).broadcast_to((128, nh, 64))
                    ta = a_stat.tile([128, nh * 64], F32, tag="ta")
                    tb = a_stat.tile([128, nh * 64], F32, tag="tb")
                    tav = ta[:].rearrange("p (h s) -> p h s", h=nh)
                    tbv = tb[:].rearrange("p (h s) -> p h s", h=nh)
                    nc.vector.tensor_mul(tav, u1, cob)
                    nc.vector.tensor_mul(tbv, u2, sib)
                    nc.vector.tensor_add(o1, tav, tbv)
                    nc.vector.tensor_mul(tav, u2, cob)
                    nc.vector.tensor_mul(tbv, u1, sib)
                    nc.vector.tensor_sub(o2, tav, tbv)
                    for j in range(nh):
                        nc.vector.tensor_scalar_mul(
                            natq[:, j * 128:(j + 1) * 128],
                            natq[:, j * 128:(j + 1) * 128], rq[:, j:j + 1])

                    # V eviction (natural layout, bf16)
                    nc.vector.tensor_copy(
                        v_sb[:, st * NKV_L * HD:(st + 1) * NKV_L * HD],
                        qkv_ps[:, NH_L * 128 + NKV_L * 128:])

                    prev = (st, natq)
                emit_transposes(*prev)

            # ---------------- stage B: attention per head
            # S^T blocks with causal column restriction (no additive mask);
            # diagonal-pair P tiles keep permanently-zeroed invalid regions,
            # a bf16 triangle mask handles the exact-diagonal 128x128
            # sub-blocks. Softmax denominators accumulate in PSUM via one
            # ones-column matmul per pair; 1/den rows are broadcast across
            # partitions on the (otherwise idle) GpSimd engine.
            # The stage-C weight pool opens here so the first projection
            # weight tiles stream in during attention.
            c_sbuf_cm = tc.tile_pool(name="c_sbuf", bufs=3)
            c_sbuf = c_sbuf_cm.__enter__()
            wp_pre = []
            for ot in range(3):
                wp_t = c_sbuf.tile([128, NH_L * 128], BF16, tag="wp")
                nc.sync.dma_start(
                    wp_t[:], wp2[:, ot * NH_L * 128:(ot + 1) * NH_L * 128])
                wp_pre.append(wp_t)
            with tc.tile_pool(name="b_sbuf", bufs=3) as b_sbuf, \
                 tc.tile_pool(name="b_diag", bufs=3) as b_diag, \
                 tc.tile_pool(name="b_row", bufs=3) as b_row, \
                 tc.tile_pool(name="s_psum", bufs=2, space="PSUM") as s_psum, \
                 tc.tile_pool(name="y_psum", bufs=3, space="PSUM") as y_psum, \
                 tc.tile_pool(name="d_psum", bufs=1, space="PSUM") as d_psum:
                # pre-zero the never-written regions of the diagonal-pair
                # tiles (ring of 3 per tag); exp only writes the causally
                # valid columns afterwards, so the zeros persist.
                for _ in range(3):
                    t0 = b_diag.tile([128, 1024], BF16, tag="pd0")
                    nc.gpsimd.memset(t0[:, 512:640], 0.0)
                    t1 = b_diag.tile([128, 1024], BF16, tag="pd1")
                    nc.gpsimd.memset(t1[:, 0:256], 0.0)
                    nc.gpsimd.memset(t1[:, 512:896], 0.0)

                for j in range(NH_L):
                    jj = j // (NH_L // NKV_L)
                    for qt in range(N_QT):
                        nblk = 4 * qt + 4
                        npair = nblk // 2
                        q0 = j * SEQ + qt * 512
                        q_sl = slice(q0, q0 + 512)
                        y_ps = y_psum.tile([128, 512], F32, tag="y")
                        dn = d_psum.tile([1, 512], F32, tag="dn")

                        def emit_y(pair):
                            g0, pp = pair
                            for half in range(2):
                                kb = 2 * g0 + half
                                nc.tensor.matmul(
                                    y_ps[:],
                                    v_sb[:, kb * NKV_L * HD + jj * HD:
                                         kb * NKV_L * HD + (jj + 1) * HD],
                                    pp[:, half * 512:(half + 1) * 512],
                                    start=(kb == 0), stop=(kb == nblk - 1))

                        pend = None
                        for g in range(npair):
                            diag = g >= 2 * qt
                            if diag:
                                p_bf = b_diag.tile(
                                    [128, 1024], BF16, tag=f"pd{g - 2 * qt}")
                            else:
                                p_bf = b_sbuf.tile([128, 1024], BF16, tag="p")
                            s_ps = s_psum.tile([128, 1024], F32, tag="s")
                            for half in range(2):
                                kb = 2 * g + half
                                d = kb - 4 * qt
                                vo = max(d, 0) * 128
                                nc.tensor.matmul(
                                    s_ps[:, half * 512 + vo:(half + 1) * 512],
                                    kt_sb[:, jj * SEQ + kb * 128:
                                          jj * SEQ + (kb + 1) * 128],
                                    qt_sb[:, q0 + vo:q0 + 512],
                                    start=True, stop=True)
                            if not diag:
                                nc.scalar.activation(p_bf[:], s_ps[:], AF.Exp)
                            else:
                                for half in range(2):
                                    d = 2 * g + half - 4 * qt
                                    vo = d * 128
                                    sl = slice(half * 512 + vo, (half + 1) * 512)
                                    nc.scalar.activation(
                                        p_bf[:, sl], s_ps[:, sl], AF.Exp)
                                for half in range(2):
                                    d = 2 * g + half - 4 * qt
                                    sl = slice(half * 512 + d * 128,
                                               half * 512 + d * 128 + 128)
                                    nc.vector.tensor_mul(
                                        p_bf[:, sl], p_bf[:, sl], mtri_sb[:])
                            tmp = b_sbuf.tile([128, 512], BF16, tag="tmp")
                            nc.vector.tensor_add(
                                tmp[:], p_bf[:, :512], p_bf[:, 512:])
                            nc.tensor.matmul(
                                dn[:], ones_col[:], tmp[:],
                                start=(g == 0), stop=(g == npair - 1))
                            if pend is not None:
                                emit_y(pend)
                            pend = (g, p_bf)
                        emit_y(pend)

                        rinv = b_row.tile([1, 512], F32, tag="rinv")
                        nc.vector.reciprocal_approx_fast(rinv[:], dn[:])
                        rb_sb = b_sbuf.tile([128, 512], F32, tag="rbs")
                        nc.gpsimd.partition_broadcast(rb_sb[:], rinv[:])
                        nc.vector.tensor_mul(
                            yt_sb[:, q_sl], y_ps[:], rb_sb[:])

            # ---------------- stage C: output projection
            with tc.tile_pool(name="c_psum", bufs=2, space="PSUM") as c_psum:
                for ot in range(DIM // 128):
                    if ot < 3:
                        wp_t = wp_pre[ot]
                    else:
                        wp_t = c_sbuf.tile([128, NH_L * 128], BF16, tag="wp")
                        nc.sync.dma_start(
                            wp_t[:],
                            wp2[:, ot * NH_L * 128:(ot + 1) * NH_L * 128])
                    po_ps = c_psum.tile([128, SEQ], F32, tag="po")
                    for ic in range(NH_L):
                        for sc in range(SEQ // 512):
                            nc.tensor.matmul(
                                po_ps[:, sc * 512:(sc + 1) * 512],
                                wp_t[:, ic * 128:(ic + 1) * 128],
                                yt_sb[:, ic * SEQ + sc * 512: ic * SEQ + (sc + 1) * 512],
                                start=(ic == 0), stop=(ic == NH_L - 1))
                    o_sb = c_sbuf.tile([128, SEQ], F32, tag="osb")
                    for sc in range(SEQ // 512):
                        nc.scalar.copy(
                            o_sb[:, sc * 512:(sc + 1) * 512],
                            po_ps[:, sc * 512:(sc + 1) * 512])
                    # output stores ride the Activation queue, parallel to
                    # the wp weight loads on the SP queue
                    nc.scalar.dma_start(out[ot * 128:(ot + 1) * 128, :], o_sb[:])
            c_sbuf_cm.__exit__(None, None, None)

    nc.compile()
    return nc


def _get_nc():
    global _CACHED_NC
    if _CACHED_NC is None:
        _CACHED_NC = _build_nc()
    return _CACHED_NC


def _make_rope_tables():
    inv_freq = 1.0 / (10000.0 ** (np.arange(0, HD, 2, dtype=np.float32) / HD))
    t = np.arange(SEQ, dtype=np.float32)
    freqs = np.outer(t, inv_freq)
    return np.concatenate(
        [np.cos(freqs), np.sin(freqs)], axis=1).astype(np.float32)


def _make_mtri():
    # lower-triangle-inclusive binary mask for the exact-diagonal block:
    # valid when query column >= key partition
    col = np.arange(128)[None, :]
    row = np.arange(128)[:, None]
    return np.ascontiguousarray(
        (col >= row).astype(ml_dtypes.bfloat16))


def _prep_in_maps(x, Wq, Wk, Wv, Wproj, q_gain):
    x = np.asarray(x, dtype=np.float32)
    Wq = np.asarray(Wq, dtype=np.float32)
    Wk = np.asarray(Wk, dtype=np.float32)
    Wv = np.asarray(Wv, dtype=np.float32)
    Wproj = np.asarray(Wproj, dtype=np.float32)
    q_gain = np.asarray(q_gain, dtype=np.float32)

    bf = ml_dtypes.bfloat16
    cs = _make_rope_tables()
    mtri = _make_mtri()
    xT = [np.ascontiguousarray(x[b].T).astype(bf) for b in range(BATCH)]
    wt_h, wp_h, g_h = [], [], []
    for h in range(2):
        w = np.concatenate([
            Wq[1024 * h:1024 * (h + 1)],
            Wk[256 * h:256 * (h + 1)],
            Wv[256 * h:256 * (h + 1)]], axis=0)
        wt_h.append(np.ascontiguousarray(w.T).astype(bf))
        wpT = Wproj[:, 1024 * h:1024 * (h + 1)].T  # [1024, 2048]
        wp_h.append(np.ascontiguousarray(
            wpT.reshape(NH_L, 128, DIM // 128, 128)
               .transpose(1, 2, 0, 3)
               .reshape(128, (DIM // 128) * NH_L * 128)).astype(bf))
        g = np.concatenate([
            q_gain[8 * h:8 * (h + 1)] / np.sqrt(HD),
            np.ones(NKV_L, np.float32)]).astype(np.float32)
        g_h.append(np.ascontiguousarray(
            np.broadcast_to(g[None, :], (128, NH_L + NKV_L))))

    in_maps = []
    for c in range(N_CORES):
        b, h = c // 2, c % 2
        in_maps.append({
            "xT": xT[b], "wt": wt_h[h], "wp2": wp_h[h], "cs": cs,
            "gains": g_h[h], "mtri": mtri,
        })
    return in_maps


def kernel(x, Wq, Wk, Wv, Wproj, q_gain):
    in_maps = _prep_in_maps(x, Wq, Wk, Wv, Wproj, q_gain)
    nc = _get_nc()
    res = run_bass_kernel_spmd(nc, in_maps, list(range(N_CORES))).results

    out = np.empty((BATCH, SEQ, DIM), dtype=np.float32)
    for b in range(BATCH):
        out[b] = (res[2 * b]["out"] + res[2 * b + 1]["out"]).T
    return out

